# revision 6
# baseline (speedup 1.0000x reference)
"""Trainium2 Bass kernel for nn_FLAttention (B=64, D=512, H=8).

Math (per batch b, head h), with xa = x*sem_w + sem_b:
    qv_q = alpha_q[h]*xa_q + beta_q[h],  kv_k = alpha_k[h]*xa_k
    r_{q,k} = 1/|kv_k - qv_q|            (eps=1e-8 negligible, folded away)
    m_q = max_k r ; e = exp(r - m_q) ; Z_q = sum_k e ; N_q = sum_k e*xa_k
    out_q = xa_q + sum_h (alpha_v[h]/sqrt(H)) * N_q/Z_q + sum_h beta_v[h]/sqrt(H)
(The beta_v term is constant because softmax rows sum to 1.)

Sharding: pure data parallel, 8 batches per core across 8 cores.

Device layout per (b,h): partitions = q (4 tiles of 128), free = k (512).
Per tile: ScalarE Abs (affine folded into per-partition scale/bias),
DVE reciprocal_approx_fast, DVE max-reduce (negated), ScalarE Exp
(bias=-max, accum_out -> Z), DVE tensor_tensor_reduce (e*xa -> N).
All per-head/per-batch constants are precomputed host-side in
partition-major layouts so the device never broadcasts or transposes
anything except one tiny PE transpose of the final [128,32] result.
"""

import math
import numpy as np
from contextlib import ExitStack

B, D, H = 64, 512, 8
NCORES = 8
BPC = B // NCORES      # batches per core = 8
P = 128                # partitions
QT = D // P            # q tiles per batch = 4
SQH = math.sqrt(H)

_PROGRAM = None


def _build_program():
    import concourse.bass as bass
    import concourse.tile as tile
    from concourse import bacc, masks, mybir

    fp32 = mybir.dt.float32
    nc = bacc.Bacc("TRN2", target_bir_lowering=False, debug=False)

    xrow_d = nc.dram_tensor("xrow", [1, BPC * D], fp32, kind="ExternalInput").ap()
    qbt_d = nc.dram_tensor("qbt", [P, BPC * H * QT], fp32, kind="ExternalInput").ap()
    skp_d = nc.dram_tensor("skp", [P, H], fp32, kind="ExternalInput").ap()
    avp_d = nc.dram_tensor("avp", [P, H * QT], fp32, kind="ExternalInput").ap()
    xap_d = nc.dram_tensor("xap", [P, BPC * QT], fp32, kind="ExternalInput").ap()
    out_d = nc.dram_tensor("out", [BPC * QT, P], fp32, kind="ExternalOutput").ap()

    A = mybir.ActivationFunctionType
    ALU = mybir.AluOpType

    with tile.TileContext(nc) as tc, ExitStack() as ctx:
        const = ctx.enter_context(tc.tile_pool(name="const", bufs=1))
        psum = ctx.enter_context(
            tc.tile_pool(name="psum", bufs=2, space=bass.MemorySpace.PSUM)
        )
        psum_out = ctx.enter_context(
            tc.tile_pool(name="psum_out", bufs=1, space=bass.MemorySpace.PSUM)
        )
        work = ctx.enter_context(tc.tile_pool(name="work", bufs=3))
        nz = ctx.enter_context(tc.tile_pool(name="nz", bufs=2))

        ones = const.tile([1, P], fp32)
        nc.gpsimd.memset(ones[:], 1.0)
        ident = const.tile([P, P], fp32)
        masks.make_identity(nc, ident[:])

        xrow = const.tile([1, BPC * D], fp32)
        nc.gpsimd.dma_start(xrow[:], xrow_d[:])
        qbt = const.tile([P, BPC * H * QT], fp32)
        nc.gpsimd.dma_start(qbt[:], qbt_d[:])
        skp = const.tile([P, H], fp32)
        nc.gpsimd.dma_start(skp[:], skp_d[:])
        avp = const.tile([P, H * QT], fp32)
        nc.gpsimd.dma_start(avp[:], avp_d[:])
        xap = const.tile([P, BPC * QT], fp32)
        nc.gpsimd.dma_start(xap[:], xap_d[:])

        # Final per-q results, columns (j*QT+qt); transposed once at the end.
        outp = const.tile([P, BPC * QT], fp32)

        for j in range(BPC):
            # XB[p, f] = xa[b, f] on every partition p (PE outer product).
            xb = psum.tile([P, D], fp32)
            nc.tensor.matmul(
                xb[:], ones[:], xrow[0:1, j * D : (j + 1) * D], start=True, stop=True
            )
            # SBUF copy for GPSIMD (which cannot read PSUM).
            xbs = work.tile([P, D], fp32, tag="xbs")
            nc.scalar.copy(xbs[:], xb[:])
            z32 = nz.tile([P, H * QT], fp32)
            n32 = nz.tile([P, H * QT], fp32)
            for h in range(H):
                rs = []
                dmin4 = work.tile([P, QT], fp32, tag="dmin4")
                for qt in range(QT):
                    col = (j * H + h) * QT + qt
                    u = work.tile([P, D], fp32)
                    nc.scalar.activation(
                        u[:],
                        xb[:],
                        A.Abs,
                        bias=qbt[:, col : col + 1],
                        scale=skp[:, h : h + 1],
                    )
                    # u2 = max(u, eps) elementwise; dmin = min_k u2 (row min)
                    u2 = work.tile([P, D], fp32)
                    nc.vector.tensor_scalar(
                        out=u2[:],
                        in0=u[:],
                        scalar1=1e-8,
                        scalar2=3.0e38,
                        op0=ALU.max,
                        op1=ALU.min,
                        accum_out=dmin4[:, qt : qt + 1],
                    )
                    r = work.tile([P, D], fp32, tag=f"r{qt}")
                    nc.vector.reciprocal_approx_fast(r[:], u2[:])
                    rs.append(r)
                # row max of r == recip(dmin) elementwise-exactly; negate for bias
                mneg4 = work.tile([P, QT], fp32, tag="mneg4")
                nc.vector.reciprocal_approx_fast(mneg4[:], dmin4[:])
                nc.vector.tensor_scalar_mul(mneg4[:], mneg4[:], -1.0)
                for qt in range(QT):
                    zc = qt * H + h
                    e = work.tile([P, D], fp32, tag=f"e{qt}")
                    nc.scalar.activation(
                        e[:],
                        rs[qt][:],
                        A.Exp,
                        bias=mneg4[:, qt : qt + 1],
                        scale=1.0,
                        accum_out=z32[:, zc : zc + 1],
                    )
                    en = work.tile([P, D], fp32, tag=f"en{qt}")
                    nc.gpsimd.tensor_mul(en[:], e[:], xbs[:])
                    nc.vector.tensor_reduce(
                        n32[:, zc : zc + 1],
                        en[:],
                        axis=mybir.AxisListType.X,
                        op=ALU.add,
                    )
            # combine: out_q = xa_q + cbeta + sum_h avp * N/Z
            rz = nz.tile([P, H * QT], fp32)
            nc.vector.reciprocal(rz[:], z32[:])
            ratio = nz.tile([P, H * QT], fp32)
            nc.vector.tensor_mul(ratio[:], n32[:], rz[:])
            scaled = nz.tile([P, H * QT], fp32)
            nc.vector.tensor_mul(scaled[:], ratio[:], avp[:])
            acc = nz.tile([P, QT], fp32)
            nc.vector.tensor_reduce(
                acc[:],
                scaled[:].rearrange("p (qt h) -> p qt h", qt=QT, h=H),
                axis=mybir.AxisListType.X,
                op=ALU.add,
            )
            nc.vector.tensor_add(
                outp[:, j * QT : (j + 1) * QT],
                acc[:],
                xap[:, j * QT : (j + 1) * QT],
            )

        outt = psum_out.tile([BPC * QT, P], fp32)
        nc.tensor.transpose(outt[:], outp[:], ident[:])
        outsb = const.tile([BPC * QT, P], fp32)
        nc.scalar.copy(outsb[:], outt[:])
        nc.gpsimd.dma_start(out_d[:], outsb[:])

    nc.compile()
    return nc


def _get_program():
    global _PROGRAM
    if _PROGRAM is None:
        _PROGRAM = _build_program()
    return _PROGRAM


def _make_in_maps(x, alpha_q, alpha_k, alpha_v, beta_q, beta_v, sem_w, sem_b):
    f = np.float32
    x = np.asarray(x, f)
    aq = np.asarray(alpha_q, f).reshape(H)
    ak = np.asarray(alpha_k, f).reshape(H)
    av = np.asarray(alpha_v, f).reshape(H)
    bq = np.asarray(beta_q, f).reshape(H)
    bv = np.asarray(beta_v, f).reshape(H)
    sw = np.asarray(sem_w, f).reshape(D)
    sb = np.asarray(sem_b, f).reshape(D)

    xa = x * sw + sb  # [B, D]
    cbeta = bv.sum() / SQH

    skp = np.tile(ak, (P, 1)).astype(f)  # [P, H]
    avp = np.zeros((P, H * QT), f)
    for qt in range(QT):
        for h in range(H):
            avp[:, qt * H + h] = av[h] / SQH

    in_maps = []
    for c in range(NCORES):
        bs = slice(c * BPC, (c + 1) * BPC)
        xa_c = xa[bs]  # [BPC, D]
        # xa in partition-major per (j, qt): [P, j, qt]
        xa_pm = xa_c.reshape(BPC, QT, P).transpose(2, 0, 1)  # [P, BPC, QT]
        qbt = np.empty((P, BPC, H, QT), f)
        for h in range(H):
            qbt[:, :, h, :] = -(aq[h] * xa_pm + bq[h])
        xap = (xa_pm + cbeta).reshape(P, BPC * QT).astype(f)
        in_maps.append(
            {
                "xrow": np.ascontiguousarray(xa_c.reshape(1, BPC * D)),
                "qbt": np.ascontiguousarray(qbt.reshape(P, BPC * H * QT)),
                "skp": skp,
                "avp": avp,
                "xap": np.ascontiguousarray(xap),
            }
        )
    return in_maps


def _assemble(results):
    f = np.float32
    out = np.empty((B, D), f)
    for c in range(NCORES):
        o = np.asarray(results[c]["out"], f)  # [BPC*QT, P]
        o = o.reshape(BPC, QT, P).reshape(BPC, D)
        out[c * BPC : (c + 1) * BPC] = o
    return out


def kernel(x, alpha_q, alpha_k, alpha_v, beta_q, beta_v, sem_w, sem_b):
    from concourse.bass_utils import run_bass_kernel_spmd

    in_maps = _make_in_maps(
        x, alpha_q, alpha_k, alpha_v, beta_q, beta_v, sem_w, sem_b
    )
    nc = _get_program()
    res = run_bass_kernel_spmd(nc, in_maps, core_ids=list(range(NCORES)))
    return _assemble(res.results)


def kernel_sim(x, alpha_q, alpha_k, alpha_v, beta_q, beta_v, sem_w, sem_b, core=0):
    """CoreSim (no hardware) single-core check: returns that core's 8 batches."""
    from concourse.bass_interp import CoreSim

    in_maps = _make_in_maps(
        x, alpha_q, alpha_k, alpha_v, beta_q, beta_v, sem_w, sem_b
    )
    nc = _get_program()
    sim = CoreSim(nc, trace=False)
    for name, arr in in_maps[core].items():
        sim.tensor(name)[:] = arr
    sim.simulate(check_with_hw=False)
    o = np.asarray(sim.tensor("out"), np.float32)
    return o.reshape(BPC, QT, P).reshape(BPC, D)
